# revision 3
# baseline (speedup 1.0000x reference)
"""Masked linear (CantorLinear): y = x @ (weight*mask).T + bias.

Structure exploited: the Cantor mask keeps ~3.9% of weights, arranged as 256
contiguous runs in the flattened (out, in) index space. Only 240 of the 2048
output rows have any nonzero weight. So the kernel packs those rows into a
[256, 2048] compact weight, computes the compact matmul on 8 NeuronCores
(data-parallel over the 16384 sequence positions), and scatters the 240
computed columns into a bias-broadcast full output on the host.

Device kernel (per core): y_cT[256, S_SH] = W_cT.T @ x_T (+ bias per row),
with K = in_features on SBUF partitions for both operands.
"""

import os
import numpy as np

import concourse.bacc as bacc
import concourse.mybir as mybir
import concourse.tile as tile
from concourse.bass_utils import run_bass_kernel_spmd

B, SQ = 4, 4096
IN_F = 2048
OUT_F = 2048
S = B * SQ                 # 16384 flattened sequence positions
NCORES = 8
S_SH = S // NCORES         # 2048 per core
R_PAD = 256                # compact out-rows padded (240 real)
P = 128
KS = IN_F // P             # 16 k-subtiles
NT = 512                   # sequence-tile width (PSUM bank = 512 fp32)
MB = R_PAD // P            # 2 output partition blocks

# matmul input dtype: "f32r" (full-rate fp32 replicated mode), "f32", "bf16"
MM_MODE = os.environ.get("CANTOR_MM_MODE", "f32r")
# repeat the whole kernel body LOOPS times inside one NEFF (benchmarking only)
LOOPS = int(os.environ.get("CANTOR_BENCH_LOOPS", "1"))

LAST_RESULTS = None  # BassKernelResults of the most recent run (for test.py)

_NC_CACHE = {}


def _build_nc(mm_mode: str, loops: int):
    io_dt = mybir.dt.bfloat16 if mm_mode == "bf16" else mybir.dt.float32
    mm_cast = {
        "f32r": mybir.dt.float32r,
        "f32": mybir.dt.float32,
        "bf16": mybir.dt.bfloat16,
    }[mm_mode]

    nc = bacc.Bacc("TRN2", target_bir_lowering=False, debug=False)
    xt = nc.dram_tensor("xt", [IN_F, S_SH], io_dt, kind="ExternalInput")
    wt = nc.dram_tensor("wt", [IN_F, R_PAD], io_dt, kind="ExternalInput")
    bc = nc.dram_tensor("bc", [R_PAD], mybir.dt.float32, kind="ExternalInput")
    yt = nc.dram_tensor("yt", [R_PAD, S_SH], mybir.dt.float32, kind="ExternalOutput")

    xt_r = xt.rearrange("(ko p) s -> p ko s", p=P)
    wt_r = wt.rearrange("(ko p) r -> p ko r", p=P)
    bc_r = bc.rearrange("(m p) -> p m", p=P)

    with tile.TileContext(nc) as tc:
        is_f32r = mm_cast == mybir.dt.float32r
        with (
            tc.tile_pool(name="wpool", bufs=1) as wpool,
            tc.tile_pool(name="xpool", bufs=2) as xpool,
            tc.tile_pool(name="opool", bufs=4) as opool,
            tc.tile_pool(name="pspool", bufs=4, space="PSUM") as pspool,
        ):
            w_ld = wpool.tile([P, KS, R_PAD], io_dt)
            nc.sync.dma_start(w_ld[:], wt_r)
            b_sb = wpool.tile([P, MB], mybir.dt.float32)
            nc.sync.dma_start(b_sb[:], bc_r)
            if is_f32r:
                # fp32r matmul inputs must come from a rounding instruction.
                w_sb = wpool.tile([P, KS, R_PAD], mybir.dt.float32r)
                nc.vector.tensor_copy(w_sb[:], w_ld[:])
            else:
                w_sb = w_ld

            def body(_i=None):
                for si in range(S_SH // NT):
                    x_ld = xpool.tile([P, KS, NT], io_dt, tag="xld")
                    nc.sync.dma_start(x_ld[:], xt_r[:, :, si * NT:(si + 1) * NT])
                    if is_f32r:
                        x_sb = xpool.tile([P, KS, NT], mybir.dt.float32r, tag="xr")
                        nc.vector.tensor_copy(x_sb[:], x_ld[:])
                    else:
                        x_sb = x_ld
                    for m in range(MB):
                        ps = pspool.tile([P, NT], mybir.dt.float32, tag="ps")
                        for k in range(KS):
                            nc.tensor.matmul(
                                ps[:],
                                lhsT=w_sb[:, k, m * P:(m + 1) * P],
                                rhs=x_sb[:, k, :],
                                start=(k == 0),
                                stop=(k == KS - 1),
                            )
                        o_sb = opool.tile([P, NT], mybir.dt.float32, tag="o")
                        nc.scalar.activation(
                            o_sb[:], ps[:],
                            mybir.ActivationFunctionType.Identity,
                            bias=b_sb[:, m:m + 1],
                        )
                        nc.sync.dma_start(
                            yt[m * P:(m + 1) * P, si * NT:(si + 1) * NT], o_sb[:]
                        )

            if loops == 1:
                body()
            else:
                with tc.For_i(0, loops, 1) as i:
                    body(i)

    nc.compile()
    return nc


def _get_nc(mm_mode: str, loops: int):
    key = (mm_mode, loops)
    if key not in _NC_CACHE:
        _NC_CACHE[key] = _build_nc(mm_mode, loops)
    return _NC_CACHE[key]


def kernel(x, weight, bias, mask):
    global LAST_RESULTS
    x = np.asarray(x, dtype=np.float32)
    weight = np.asarray(weight, dtype=np.float32)
    bias = np.asarray(bias, dtype=np.float32)
    mask = np.asarray(mask, dtype=np.float32)

    w_eff = weight * mask
    rows = np.flatnonzero(mask.any(axis=1))
    r = len(rows)
    assert r <= R_PAD, f"compact rows {r} > padded {R_PAD}"

    if MM_MODE == "bf16":
        import ml_dtypes
        io_np = ml_dtypes.bfloat16
    else:
        io_np = np.float32

    w_c = np.zeros((R_PAD, IN_F), dtype=np.float32)
    w_c[:r] = w_eff[rows]
    wt = np.ascontiguousarray(w_c.T).astype(io_np)      # [IN_F, R_PAD]
    bc = np.zeros((R_PAD,), dtype=np.float32)
    bc[:r] = bias[rows]

    xf = x.reshape(S, IN_F)
    in_maps = []
    for c in range(NCORES):
        x_t = np.ascontiguousarray(xf[c * S_SH:(c + 1) * S_SH].T).astype(io_np)
        in_maps.append({"xt": x_t, "wt": wt, "bc": bc})

    nc = _get_nc(MM_MODE, LOOPS)
    res = run_bass_kernel_spmd(nc, in_maps, list(range(NCORES)))
    LAST_RESULTS = res

    y = np.empty((S, OUT_F), dtype=np.float32)
    y[:] = bias
    for c in range(NCORES):
        y[c * S_SH:(c + 1) * S_SH, rows] = res.results[c]["yt"][:r].T
    return y.reshape(B, SQ, OUT_F)
